# revision 1
# baseline (speedup 1.0000x reference)
"""Trainium2 Bass kernel for relative-position multi-head attention.

Problem shape (hardcoded): B=2, T=1024, CH=1024, HEADS=16, KC=64, WIN=4.
Sharding: tensor-parallel over heads across 8 cores (2 heads/core),
AllGather of head outputs, per-core column shard of the output projection.

Key observation: with T=1024 and window 4, the relative-position logits
(_rel_to_abs) and weights (_abs_to_rel) only touch the 9 diagonals
|j - i| <= 4 of the [T, T] score matrix.  Both the band-add (scores) and
the band-gather (rel_v epilogue) are routed through a small DRAM staging
buffer with a "diagonal-compact" layout: value for (j, i) at flat
(j+4)*144 + (i-j+4).  Rectangular SBUF windows [128, 136] of the [j, i]
plane map to [[143, 128], [1, 136]] patterns there (partition-outer,
contiguous inner runs), and unused slots m in [9, 144) are zeroed once so
out-of-band window cells read zeros.
"""

import sys

sys.path.insert(0, "/opt/trn_rl_repo")

import math
import numpy as np

import concourse.bass as bass
import concourse.tile as tile
from concourse import mybir
from concourse import bacc
from concourse.bass_utils import run_bass_kernel_spmd
from concourse.masks import make_identity

# ---------------------------------------------------------------- constants
B, T, CH, HEADS, KC, WIN = 2, 1024, 1024, 16, 64, 4
NCORES = 8
HPC = HEADS // NCORES          # heads per core = 2
DPC = HPC * KC                 # channels per core = 128
NI = B * T                     # 2048 flattened (b, t)
M9 = 2 * WIN + 1               # 9 diagonals
WSLOT = 144                    # diag-compact row stride (> 135 garbage range)
WBND = 136                     # band window width (i in [j0-4, j0+132))
EXPW = 8 * 1024 + 8            # per-unit exp(S) tensor width (+8 pad cols)
F32 = mybir.dt.float32
F32R = mybir.dt.float32r
AF = mybir.ActivationFunctionType

_CACHE = {}


# ---------------------------------------------------------------- program
def build_program():
    nc = bacc.Bacc("TRN2", target_bir_lowering=False, debug=False,
                   num_devices=NCORES)

    xT = nc.dram_tensor("xT", [CH, NI], F32R, kind="ExternalInput")
    cT = nc.dram_tensor("cT", [CH, NI], F32R, kind="ExternalInput")
    wq = nc.dram_tensor("wq", [CH, DPC], F32R, kind="ExternalInput")
    wk = nc.dram_tensor("wk", [CH, DPC], F32R, kind="ExternalInput")
    wv = nc.dram_tensor("wv", [CH, DPC], F32R, kind="ExternalInput")
    wo = nc.dram_tensor("wo", [CH, DPC], F32R, kind="ExternalInput")
    bq = nc.dram_tensor("bq", [DPC, 1], F32, kind="ExternalInput")
    bk = nc.dram_tensor("bk", [DPC, 1], F32, kind="ExternalInput")
    bv = nc.dram_tensor("bv", [DPC, 1], F32, kind="ExternalInput")
    bo = nc.dram_tensor("bo", [DPC, 1], F32, kind="ExternalInput")
    erk = nc.dram_tensor("erk", [DPC, M9], F32R, kind="ExternalInput")
    erv = nc.dram_tensor("erv", [M9, KC + 1], F32R, kind="ExternalInput")
    zros = nc.dram_tensor("zros", [128, 1164], F32, kind="ExternalInput")
    one8 = nc.dram_tensor("one8", [128, 8], F32R, kind="ExternalInput")
    outT = nc.dram_tensor("outT", [DPC, NI], F32, kind="ExternalOutput")

    with tile.TileContext(nc) as tc:
        with (
            tc.tile_pool(name="const", bufs=1) as cpool,
            tc.tile_pool(name="persist", bufs=1) as ppool,
            tc.tile_pool(name="dram", bufs=1, space="DRAM") as dpool,
        ):
            # ---------------- constants / weights to SBUF (1 DMA each)
            wsb = {}
            for nm, src in (("wq", wq), ("wk", wk), ("wv", wv), ("wo", wo)):
                t_ = cpool.tile([128, 8 * DPC], F32R, name=f"{nm}_sb")
                # chunk d8 at free cols [128*d8, 128*(d8+1)): src row 128*d8+p
                sap = bass.AP(src, 0, [[DPC, 128], [DPC * 128, 8], [1, DPC]])
                nc.scalar.dma_start(
                    t_[:].rearrange("p (c8 c) -> p c8 c", c=DPC), sap)
                wsb[nm] = t_

            def wtile(nm, d8):
                return wsb[nm][:, DPC * d8:DPC * (d8 + 1)]

            bq_sb = cpool.tile([DPC, 1], F32, name="bq_sb")
            bk_sb = cpool.tile([DPC, 1], F32, name="bk_sb")
            bv_sb = cpool.tile([DPC, 1], F32, name="bv_sb")
            bo_sb = cpool.tile([DPC, 1], F32, name="bo_sb")
            for t_, src in ((bq_sb, bq), (bk_sb, bk), (bv_sb, bv), (bo_sb, bo)):
                nc.scalar.dma_start(t_[:], src[:])
            erk_sb = cpool.tile([DPC, M9], F32R, name="erk_sb")
            nc.scalar.dma_start(erk_sb[:], erk[:])
            erv_sb = cpool.tile([M9, KC + 1], F32R, name="erv_sb")
            nc.scalar.dma_start(erv_sb[:], erv[:])
            ident = cpool.tile([128, 128], F32, name="ident")
            make_identity(nc, ident[:])

            # persistent activations
            qT_sb = ppool.tile([DPC, NI], F32R, name="qT_sb")
            kT_sb = ppool.tile([DPC, NI], F32R, name="kT_sb")
            vT_sb = ppool.tile([DPC, NI], F32, name="vT_sb")
            OT_sb = ppool.tile([DPC, NI], F32R, name="OT_sb")
            # v in [j, d] layout + ones column, per unit (b, h): [128, 8*65]
            vju = [ppool.tile([128, 8 * (KC + 1)], F32R, name=f"vju{u}")
                   for u in range(4)]

            # DRAM staging for the diagonal band (double buffered by parity)
            rd_d = [dpool.tile([128 * 1164], F32, name=f"rd{p}")
                    for p in range(2)]
            gd_d = [dpool.tile([(T + 8) * WSLOT], F32R, name=f"gd{p}")
                    for p in range(2)]
            ag_in = dpool.tile([DPC, NI], F32R, name="ag_in")
            ag_out = dpool.tile([NCORES * DPC, NI], F32R, name="ag_out",
                                addr_space="Shared")

            # zero the Rd buffers once (slots >= 9 and j-header/tail stay 0)
            for p in range(2):
                flat = rd_d[p][:].rearrange("(r c) -> r c", c=1164)
                nc.sync.dma_start(flat[:], zros[:])
                # Gd: only the j-invalid edge rows must be zero (read as
                # the clipped rel_w corners); windows never write them.
                flatg = gd_d[p][:].rearrange("(r c) -> r c", c=WSLOT)
                nc.sync.dma_start(flatg[0:4, :],
                                  zros[:4, :WSLOT].bitcast(F32R))
                nc.sync.dma_start(flatg[T + 4:T + 8, :],
                                  zros[:4, :WSLOT].bitcast(F32R))
            # ones columns of vju (one strided DMA per unit)
            for u in range(4):
                dst = bass.AP(vju[u].tensor, KC,
                              [[8 * (KC + 1), 128], [KC + 1, 8]])
                nc.scalar.dma_start(dst, one8[:])

            # ---------------- phase A: QKV projections (transposed layouts)
            with (
                tc.tile_pool(name="xin", bufs=12) as xpool,
                tc.tile_pool(name="qkvps", bufs=1, space="PSUM") as qkvps,
                tc.tile_pool(name="tps", bufs=2, space="PSUM") as tpps,
            ):
                # q: load all 8 row-blocks of xT, 4 open accumulation groups
                xts = []
                for d8 in range(8):
                    t_ = xpool.tile([128, NI], F32R, tag="xt")
                    nc.gpsimd.dma_start(t_[:], xT[d8 * 128:(d8 + 1) * 128, :])
                    xts.append(t_)
                qps = [qkvps.tile([DPC, 512], F32, tag=f"proj{it}",
                                  name=f"qp{it}") for it in range(4)]
                for d8 in range(8):
                    for it in range(4):
                        nc.tensor.matmul(
                            qps[it][:], wtile("wq", d8),
                            xts[d8][:, it * 512:(it + 1) * 512],
                            start=(d8 == 0), stop=(d8 == 7))
                for it in range(4):
                    nc.vector.tensor_scalar_add(
                        qT_sb[:, it * 512:(it + 1) * 512], qps[it][:], bq_sb[:])

                cts = []
                for d8 in range(8):
                    t_ = xpool.tile([128, NI], F32R, tag="xt")
                    nc.gpsimd.dma_start(t_[:], cT[d8 * 128:(d8 + 1) * 128, :])
                    cts.append(t_)
                kps = [qkvps.tile([DPC, 512], F32, tag=f"proj{it}",
                                  name=f"kp{it}") for it in range(4)]
                for d8 in range(8):
                    for it in range(4):
                        nc.tensor.matmul(
                            kps[it][:], wtile("wk", d8),
                            cts[d8][:, it * 512:(it + 1) * 512],
                            start=(d8 == 0), stop=(d8 == 7))
                for it in range(4):
                    nc.scalar.activation(kT_sb[:, it * 512:(it + 1) * 512],
                                         kps[it][:], AF.Identity, bias=bk_sb[:])
                vps = [qkvps.tile([DPC, 512], F32, tag=f"proj{it}",
                                  name=f"vp{it}") for it in range(4)]
                for d8 in range(8):
                    for it in range(4):
                        nc.tensor.matmul(
                            vps[it][:], wtile("wv", d8),
                            cts[d8][:, it * 512:(it + 1) * 512],
                            start=(d8 == 0), stop=(d8 == 7))
                for it in range(4):
                    nc.vector.tensor_scalar_add(
                        vT_sb[:, it * 512:(it + 1) * 512], vps[it][:], bv_sb[:])

                # transpose v to [j, d] per unit; ones col already DMA'd
                for u in range(4):
                    b, h = divmod(u, 2)
                    for jc in range(8):
                        tp = tpps.tile([128, KC], F32, tag="tp")
                        nc.tensor.transpose(
                            tp[:],
                            vT_sb[64 * h:64 * h + 64,
                                  1024 * b + 128 * jc:1024 * b + 128 * (jc + 1)],
                            ident[64 * h:64 * h + 64, 64 * h:64 * h + 64])
                        nc.scalar.activation(
                            vju[u][:, 65 * jc:65 * jc + 64], tp[:], AF.Copy)

            # ---------------- phase B: attention per unit
            with (
                tc.tile_pool(name="spool", bufs=2, space="PSUM") as spool,
                tc.tile_pool(name="opool", bufs=2, space="PSUM") as opool,
                tc.tile_pool(name="bnd", bufs=2) as bndpool,
                tc.tile_pool(name="exps", bufs=2) as exppool,
                tc.tile_pool(name="misc", bufs=2) as mpool,
            ):
                for u in range(4):
                    b, h = divmod(u, 2)
                    par = u % 2
                    hb = 64 * h
                    ib = 1024 * b
                    rd_t = rd_d[par]
                    gd_t = gd_d[par]

                    # R^T[t, i] = sum_d erk[t, d] * qs[d, i]   -> [9, 1024]
                    rp = spool.tile([M9, T], F32, tag="sps", name=f"rp{u}")
                    for s in range(2):
                        nc.tensor.matmul(
                            rp[:, 512 * s:512 * (s + 1)],
                            erk_sb[hb:hb + 64, :],
                            qT_sb[hb:hb + 64, ib + 512 * s:ib + 512 * (s + 1)],
                            start=True, stop=True)
                    r_sb = mpool.tile([M9, T], F32, tag="r_sb")
                    nc.vector.tensor_copy(r_sb[:], rp[:])
                    # staircase write: r_sb[t, i] -> Rd[(i+t)*144 + (8-t)]
                    dst = bass.AP(rd_t.tensor, 8,
                                  [[WSLOT - 1, M9], [WSLOT, T]])
                    nc.sync.dma_start(dst, r_sb[:])
                    # all 8 band windows in one DMA: Bnd[p, 136*jt + c]
                    bnd = bndpool.tile([128, 8 * WBND], F32, tag="bnd")
                    srcb = bass.AP(rd_t.tensor, 4 * WSLOT,
                                   [[WSLOT - 1, 128], [128 * WSLOT, 8],
                                    [1, WBND]])
                    nc.sync.dma_start(
                        bnd[:].rearrange("p (j c) -> p j c", c=WBND), srcb)

                    expt = exppool.tile([128, EXPW], F32R, tag="expt")
                    op = [opool.tile([KC + 1, 512], F32, tag=f"ops{s}",
                                     name=f"ops{s}_{u}") for s in range(2)]

                    for jt in range(8):
                        j0 = 128 * jt
                        sp = spool.tile([128, T], F32, tag="sps")
                        for s in range(2):
                            nc.tensor.matmul(
                                sp[:, 512 * s:512 * (s + 1)],
                                kT_sb[hb:hb + 64, ib + j0:ib + j0 + 128],
                                qT_sb[hb:hb + 64,
                                      ib + 512 * s:ib + 512 * (s + 1)],
                                start=True, stop=True)
                        # band add: window i in [j0-4, j0+132), clipped
                        a = max(0, j0 - 4)
                        e = min(T, j0 + 132)
                        s0 = a - (j0 - 4)
                        nc.vector.tensor_add(
                            sp[:, a:e], sp[:, a:e],
                            bnd[:, WBND * jt + s0:WBND * jt + s0 + (e - a)])
                        ecol = 1024 * jt
                        nc.scalar.activation(expt[:, ecol:ecol + T], sp[:],
                                             AF.Exp)
                        # PV + colsum (ones column fused in vju)
                        for s in range(2):
                            nc.tensor.matmul(
                                op[s][:],
                                vju[u][:, 65 * jt:65 * (jt + 1)],
                                expt[:, ecol + 512 * s:ecol + 512 * (s + 1)],
                                start=(jt == 0), stop=False)
                    # band windows of exp(S) -> Gd (2 DMAs: jt=0, jt=1..7)
                    dst0 = bass.AP(gd_t.tensor, 4 * WSLOT + 4,
                                   [[WSLOT - 1, 128], [1, 132]])
                    nc.scalar.dma_start(dst0, expt[:, 0:132])
                    dst17 = bass.AP(gd_t.tensor, 132 * WSLOT,
                                    [[WSLOT - 1, 128], [128 * WSLOT, 7],
                                     [1, WBND]])
                    src17 = bass.AP(expt.tensor, 1148,
                                    [[EXPW, 128], [1024 + 128, 7], [1, WBND]])
                    nc.scalar.dma_start(dst17, src17)

                    # gather the 9 diagonals of exp(S): G9[t, i]
                    g9 = mpool.tile([M9, T], F32R, tag="g9")
                    srcg = bass.AP(gd_t.tensor, 8,
                                   [[WSLOT - 1, M9], [WSLOT, T]])
                    nc.scalar.dma_start(g9[:], srcg)
                    for s in range(2):
                        nc.tensor.matmul(
                            op[s][:], erv_sb[:],
                            g9[:, 512 * s:512 * (s + 1)],
                            start=False, stop=True)

                    # normalize by colsum (row KC) and write to OT
                    cs1 = mpool.tile([1, T], F32, tag="cs1")
                    rcp64 = mpool.tile([64, T], F32, tag="rcp64")
                    for s in range(2):
                        nc.scalar.activation(cs1[:, 512 * s:512 * (s + 1)],
                                             op[s][KC:KC + 1, :], AF.Copy)
                    nc.gpsimd.partition_broadcast(rcp64[:], cs1[:])
                    nc.vector.reciprocal(rcp64[:], rcp64[:])
                    for s in range(2):
                        nc.vector.tensor_mul(
                            OT_sb[hb:hb + 64, ib + 512 * s:ib + 512 * (s + 1)],
                            op[s][0:KC, :], rcp64[:, 512 * s:512 * (s + 1)])

            # ---------------- phase C: AllGather + output projection
            nc.sync.dma_start(ag_in[:], OT_sb[:])
            nc.gpsimd.collective_compute(
                "AllGather", mybir.AluOpType.bypass,
                replica_groups=[list(range(NCORES))],
                ins=[ag_in[:].opt()], outs=[ag_out[:].opt()])
            with (
                tc.tile_pool(name="cg", bufs=8) as cgpool,
                tc.tile_pool(name="fps", bufs=1, space="PSUM") as fpool,
                tc.tile_pool(name="osb", bufs=4) as opool2,
            ):
                cgs = []
                for ct in range(8):
                    t_ = cgpool.tile([128, NI], F32R, tag="cg")
                    nc.gpsimd.dma_start(t_[:],
                                        ag_out[ct * 128:(ct + 1) * 128, :])
                    cgs.append(t_)
                fps = [fpool.tile([DPC, 512], F32, tag=f"fps{it}",
                                  name=f"fp{it}") for it in range(4)]
                for ct in range(8):
                    for it in range(4):
                        nc.tensor.matmul(
                            fps[it][:], wtile("wo", ct),
                            cgs[ct][:, it * 512:(it + 1) * 512],
                            start=(ct == 0), stop=(ct == 7))
                for it in range(4):
                    sl = slice(it * 512, (it + 1) * 512)
                    ot = opool2.tile([DPC, 512], F32, tag="osb")
                    nc.vector.tensor_scalar_add(ot[:], fps[it][:], bo_sb[:])
                    nc.scalar.dma_start(outT[:, sl], ot[:])

    nc.compile()
    return nc


# ---------------------------------------------------------------- host side
def _prep_inputs(x, c, Wq, bq, Wk, bk, Wv, bv, Wo, bo, emb_rel_k, emb_rel_v):
    scale = 1.0 / math.sqrt(KC)
    xT = np.ascontiguousarray(
        x.reshape(NI, CH).T.astype(np.float32))          # [CH, NI]
    cT = np.ascontiguousarray(c.reshape(NI, CH).T.astype(np.float32))
    Wq_s = (Wq * scale).astype(np.float32)
    bq_s = (bq * scale).astype(np.float32)
    erv_p = np.concatenate(
        [emb_rel_v[0], np.zeros((M9, 1), np.float32)], axis=1)  # [9, 65]
    erk2 = np.ascontiguousarray(
        np.concatenate([emb_rel_k[0].T, emb_rel_k[0].T], axis=0))  # [128, 9]
    in_maps = []
    for cix in range(NCORES):
        sl = slice(cix * DPC, (cix + 1) * DPC)
        in_maps.append({
            "xT": xT, "cT": cT,
            "wq": np.ascontiguousarray(Wq_s[:, sl]),
            "wk": np.ascontiguousarray(Wk[:, sl].astype(np.float32)),
            "wv": np.ascontiguousarray(Wv[:, sl].astype(np.float32)),
            "wo": np.ascontiguousarray(Wo[:, sl].astype(np.float32)),
            "bq": np.ascontiguousarray(bq_s[sl, None]),
            "bk": np.ascontiguousarray(bk[sl, None].astype(np.float32)),
            "bv": np.ascontiguousarray(bv[sl, None].astype(np.float32)),
            "bo": np.ascontiguousarray(bo[sl, None].astype(np.float32)),
            "erk": erk2.astype(np.float32),
            "erv": erv_p.astype(np.float32),
            "zros": np.zeros((128, 1164), np.float32),
            "one8": np.ones((128, 8), np.float32),
        })
    return in_maps


def _numpy_fallback(x, c, mask, Wq, bq, Wk, bk, Wv, bv, Wo, bo,
                    emb_rel_k, emb_rel_v):
    # general-mask reference path (never taken for the spec'd all-ones mask)
    q = (x.reshape(NI, CH) @ Wq + bq).reshape(B, T, HEADS, KC).transpose(0, 2, 1, 3)
    k = (c.reshape(NI, CH) @ Wk + bk).reshape(B, T, HEADS, KC).transpose(0, 2, 1, 3)
    v = (c.reshape(NI, CH) @ Wv + bv).reshape(B, T, HEADS, KC).transpose(0, 2, 1, 3)
    qs = q / math.sqrt(KC)
    scores = np.einsum("bhtd,bhsd->bhts", qs, k)
    idx_j = np.arange(T)[None, :] - np.arange(T)[:, None] + WIN  # j - i + 4
    band = (idx_j >= 0) & (idx_j <= 2 * WIN)
    rel = np.einsum("bhtd,md->bhtm", qs, emb_rel_k[0])  # [B,H,T,9]
    bias = np.zeros((B, HEADS, T, T), np.float32)
    ii, jj = np.nonzero(band)
    bias[:, :, ii, jj] = rel[:, :, ii, idx_j[ii, jj]]
    scores = scores + bias
    scores = np.where(mask == 0, np.float32(1e-4), scores)
    scores -= scores.max(axis=-1, keepdims=True)
    p = np.exp(scores)
    p /= p.sum(axis=-1, keepdims=True)
    out = np.einsum("bhts,bhsd->bhtd", p, v)
    relw = np.zeros((B, HEADS, T, M9), np.float32)
    relw[:, :, ii, idx_j[ii, jj]] = p[:, :, ii, jj]
    out = out + np.einsum("bhtm,md->bhtd", relw, emb_rel_v[0])
    out = out.transpose(0, 2, 1, 3).reshape(NI, CH)
    return (out @ Wo + bo).reshape(B, T, CH).astype(np.float32)


def kernel(x, c, mask, Wq, bq, Wk, bk, Wv, bv, Wo, bo, emb_rel_k, emb_rel_v,
           _collect=None):
    x = np.asarray(x); c = np.asarray(c); mask = np.asarray(mask)
    args = [np.asarray(a) for a in
            (Wq, bq, Wk, bk, Wv, bv, Wo, bo, emb_rel_k, emb_rel_v)]
    if not np.all(mask):
        return _numpy_fallback(x, c, mask, *args)

    if "nc" not in _CACHE:
        _CACHE["nc"] = build_program()
    nc = _CACHE["nc"]

    in_maps = _prep_inputs(x, c, *args)
    res = run_bass_kernel_spmd(nc, in_maps, core_ids=list(range(NCORES)))
    if _collect is not None:
        _collect.append(res)
    out = np.empty((NI, CH), np.float32)
    for cix in range(NCORES):
        out[:, cix * DPC:(cix + 1) * DPC] = res.results[cix]["outT"].T
    return out.reshape(B, T, CH)

